# revision 38
# baseline (speedup 1.0000x reference)
"""Trainium2 Bass kernel for nn_Attention_54391465836966.

Math (per batch b, component n; one (b, n) pair per core):
  ctok = content_feat[b].reshape(S,C) + pos           # raw reshape tokens
  comp_tok = components[n,b].reshape(S,C) + pos
  q = ctok @ Wq ; k,v = comp_tok @ Wkv (split)
  per head h: P' = exp(scale*q k^T - 12); o_h = (P' @ v) / rowsum(P')
  result = sum_n o_n ; s = (result + ctok) @ Wproj + bproj
  out = Wconv[:, :512] . s2d + Wconv[:, 512:] . cf2d + bconv
    (s2d = raw [C, S] view of the token-major s buffer)

All device data is fp16 (f32 PSUM accumulation); exp carries a constant
-12 bias so probabilities fit fp16 (softmax is invariant to it).  Host
passes token-channel-major transposes (comp^T, cf^T, pos^T) and Wconv^T,
so the kernel does zero PE transposes.  Query tokens are parity-permuted
(even tokens first) end-to-end: the proj output tiles then directly ARE
the raw-reshape s2d chunks the conv needs, removing the DRAM round-trip.
The (result + ctok) constant term rides the attention output as
+0.25*ctok^T per core (host sums 4 component partials per batch); the
bproj term's conv contribution is the rank-1 update colsum(Wconv_s) x
bproj/4, folded into the cf-side conv.

Schedule: attention is software-pipelined one head deep -- block h emits
head h's scores+exp (the ACT-critical stream, split in 512-wide halves
so the 4-deep one-bank PSUM ring hides the exp->score rebuild latency)
interleaved per-kt with head h-1's o matmuls and one filler group
(v/kT/qT setup in heads 0-3, cf-conv quarters in 4-5).  Normalization
takes 1/Z via the fast DVE reciprocal off the ones-column row and
scales into head-pair tiles; proj groups open early with the three
merged pairs and close after the final merge, overlapping the h7
normalization with the remaining cf-conv and o-drain work.
"""
import sys

sys.path.insert(0, "/opt/trn_rl_repo")

import numpy as np

N_CORES = 8
B, C, H, W = 2, 512, 32, 32
S = H * W  # 1024
NH, HD = 8, 64
SCALE = HD ** -0.5
EXP_BIAS = -12.0

_CACHE = {}


def _build():
    if "nc" in _CACHE:
        return _CACHE["nc"]
    from contextlib import ExitStack

    import concourse.bacc as bacc
    import concourse.mybir as mybir
    import concourse.tile as tile

    f16 = mybir.dt.float16
    f32 = mybir.dt.float32
    EXP = mybir.ActivationFunctionType.Exp

    nc = bacc.Bacc("TRN2", target_bir_lowering=False, debug=False,
                   num_devices=N_CORES)

    din = lambda n, s, dt=f16: nc.dram_tensor(n, s, dt, kind="ExternalInput").ap()
    compT_d = din("compT", [C, S])   # components[n,b] token-chan-major (host .T)
    posT_d = din("posT", [C, S])     # pos^T
    cfT_d = din("cfT", [C, S])       # content tokens^T
    wk_d = din("wk", [C, C])         # Wkv[:, :C]
    wv_d = din("wv", [C, C])         # Wkv[:, C:]
    wq_d = din("wq", [C, C])
    wproj_d = din("wproj", [C, C])
    wcs_d = din("wcs", [C, C])       # Wconv^T rows 0:C   (s part)
    wccf1_d = din("wccf1", [128, C])  # this core's c-quarter of Wconv^T cf rows
    cfc1_d = din("cfc1", [128, S])    # matching c-quarter of channel-major cf
    b2_d = din("b2", [2, C])          # rows: bconv/4, colsum(Wconv_s)
    r2_d = din("r2", [2, 512])        # rows: ones, bproj/4
    out_p = nc.dram_tensor("out_p", [C, S], f16, kind="ExternalOutput").ap()

    wview = lambda d: d.rearrange("(k p) c -> p k c", p=128)
    tview = lambda t: t[:].rearrange("p (k c) -> p k c", k=4)

    with tile.TileContext(nc) as tc, ExitStack() as ctx:
        main = ctx.enter_context(tc.tile_pool(name="main", bufs=1))
        trans = ctx.enter_context(tc.tile_pool(name="trans", bufs=2))

        # ---- input DMAs (SP ring, in order of first use) ----
        compT_raw = [main.tile([128, S], f16, tag=f"cr{j}", name=f"compTr{j}")
                     for j in range(4)]
        posT = [main.tile([128, S], f16, tag=f"pos{j}", name=f"posT{j}")
                for j in range(4)]
        cfT = [main.tile([128, S], f16, tag=f"cfT{j}", name=f"cfT{j}")
               for j in range(4)]
        for j in range(4):
            nc.sync.dma_start(compT_raw[j][:], compT_d[128 * j:128 * (j + 1), :])
            nc.sync.dma_start(posT[j][:], posT_d[128 * j:128 * (j + 1), :])
            nc.sync.dma_start(cfT[j][:], cfT_d[128 * j:128 * (j + 1), :])
        wk = main.tile([128, 4 * C], f16, tag="wk")
        wq = main.tile([128, 4 * C], f16, tag="wq")
        wv = main.tile([128, 4 * C], f16, tag="wv")
        nc.sync.dma_start(tview(wk), wview(wk_d)[:, :, :])
        nc.sync.dma_start(tview(wq), wview(wq_d)[:, :, :])
        nc.sync.dma_start(tview(wv), wview(wv_d)[:, :, :])
        wproj = main.tile([128, 4 * C], f16, tag="wp")
        wcs = main.tile([128, 4 * C], f16, tag="wcs")
        wccf1 = main.tile([128, C], f16, tag="wccf1")
        cfc1 = main.tile([128, S], f16, tag="cfc1")
        b2 = main.tile([2, C], f16, tag="b2")
        r2 = main.tile([2, 512], f16, tag="r2")
        nc.sync.dma_start(tview(wproj), wview(wproj_d)[:, :, :])
        nc.sync.dma_start(tview(wcs), wview(wcs_d)[:, :, :])
        nc.sync.dma_start(wccf1[:], wccf1_d[:])
        nc.sync.dma_start(cfc1[:], cfc1_d[:])
        nc.sync.dma_start(b2[:], b2_d[:])
        nc.sync.dma_start(r2[:], r2_d[:])

        wk_v, wq_v, wv_v = tview(wk), tview(wq), tview(wv)
        wproj_v, wcs_v = tview(wproj), tview(wcs)

        ones = main.tile([128, S], f16, tag="ones")
        nc.gpsimd.memset(ones[:], 1.0)
        ebias = main.tile([128, 1], f32, tag="ebias")
        nc.gpsimd.memset(ebias[:], EXP_BIAS)
        # pull the ACT function-table load off the critical path
        actwarm = main.tile([1, 1], f16, tag="actwarm")
        nc.scalar.activation(actwarm[0:1, 0:1], ebias[0:1, 0:1], EXP,
                             bias=ebias[0:1, 0:1], scale=SCALE)

        # ---- token adds ----
        tok = [main.tile([128, S], f16, tag=f"tok{j}", name=f"tok{j}")
               for j in range(4)]
        for j in range(4):
            nc.vector.tensor_add(tok[j][:], compT_raw[j][:], posT[j][:])
        # content tokens: parity-permuted columns (even tokens then odd)
        ctokT = [main.tile([128, S], f16, tag=f"ctokT{j}", name=f"ctokT{j}")
                 for j in range(4)]
        ev = lambda ap, par: ap.rearrange("p (a two) -> p a two", two=2)[
            :, :, par:par + 1]
        for j in range(4):
            for par in range(2):
                dst = ctokT[j][:, 512 * par:512 * (par + 1)].rearrange(
                    "p (a o) -> p a o", o=1)
                nc.vector.tensor_add(dst, ev(cfT[j][:], par), ev(posT[j][:], par))

        kT = [main.tile([128, S], f16, tag=f"kT{j}", name=f"kT{j}")
              for j in range(4)]
        qT = [main.tile([128, S], f16, tag=f"qT{j}", name=f"qT{j}")
              for j in range(4)]
        v_sb = [main.tile([128, 8 * 65], f16, tag=f"v{t}", name=f"v{t}")
                for t in range(8)]
        vv = lambda t: v_sb[t][:].rearrange("p (h e) -> p h e", h=8)
        for t in range(8):
            # ones column LAST (e=64): Z lands on o_ps partition 64 (aligned)
            nc.gpsimd.tensor_copy(vv(t)[:, :, 64:65],
                                  ones[:, 0:8].rearrange("p (h w) -> p h w", w=1))
        pair = [main.tile([128, S], f16, tag=f"pair{j}", name=f"pair{j}")
                for j in range(4)]
        outcf = [main.tile([128, S], f32, tag=f"ocf{oc}", name=f"ocf{oc}")
                 for oc in range(4)]
        s_sb = [main.tile([128, C], f16, tag=f"s{t}", name=f"s{t}")
                for t in range(8)]
        out_sb = [main.tile([128, S], f16, tag=f"ob{oc}", name=f"ob{oc}")
                  for oc in range(4)]

        with tc.tile_pool(name="ps", bufs=1, space="PSUM") as ps:
            _n = [0]

            def pst(tag, shape, bufs):
                _n[0] += 1
                return ps.tile(shape, f32, tag=tag, bufs=bufs,
                               name=f"{tag}{_n[0]}")

            sct = lambda: pst("sc", [128, 512], 4)

            # ramp the PE p-state during the DMA shadow (outputs unused)
            warm_ps = sct()
            for _ in range(28):
                nc.tensor.matmul(warm_ps[:], ones[:, 0:128],
                                 ones[:, 0:512], start=True, stop=True)

            # ---- filler emitters (run inside attention's ACT-bound gaps) --
            def emit_kq(dst, wsrc, act, j, tck, on_act=False):
                acc = sct()
                for k in range(4):
                    nc.tensor.matmul(acc[:],
                                     wsrc[:, k, 128 * j:128 * (j + 1)],
                                     act[k][:, 512 * tck:512 * (tck + 1)],
                                     start=(k == 0), stop=(k == 3))
                if on_act:
                    # lead-in only: ACT is idle before the exp stream starts
                    nc.scalar.copy(dst[j][:, 512 * tck:512 * (tck + 1)],
                                   acc[:])
                else:
                    nc.vector.tensor_copy(dst[j][:, 512 * tck:512 * (tck + 1)],
                                          acc[:])

            def emit_v(t):
                acc = sct()
                for k in range(4):
                    nc.tensor.matmul(acc[:],
                                     tok[k][:, 128 * t:128 * (t + 1)],
                                     wv_v[:, k, :],
                                     start=(k == 0), stop=(k == 3))
                nc.vector.tensor_copy(
                    vv(t)[:, :, 0:64],
                    acc[:].rearrange("p (h d) -> p h d", h=8))

            def emit_cc(oc):
                # this core's c-quarter of the cf-side conv + the rank-1
                # bias terms (bconv/4 x ones, colsum(Wcs) x bproj/4)
                for pc in range(2):
                    half = sct()
                    nc.tensor.matmul(half[:], b2[0:2, 128 * oc:128 * (oc + 1)],
                                     r2[0:2, :], start=True, stop=False)
                    nc.tensor.matmul(half[:],
                                     wccf1[:, 128 * oc:128 * (oc + 1)],
                                     cfc1[:, 512 * pc:512 * (pc + 1)],
                                     start=False, stop=True)
                    nc.vector.tensor_copy(outcf[oc][:, 512 * pc:512 * (pc + 1)],
                                          half[:])

            # ---- lead-in: kT0 / qT0 (copies on the still-idle ACT) ----
            emit_kq(kT, wk_v, tok, 0, 0, on_act=True)
            emit_kq(kT, wk_v, tok, 0, 1, on_act=True)
            emit_kq(qT, wq_v, ctokT, 0, 0, on_act=True)
            emit_kq(qT, wq_v, ctokT, 0, 1, on_act=True)

            # ---- attention, software-pipelined one head deep ----
            # Block h emits head h's scores+exp (the ACT critical stream)
            # interleaved per-kt with head h-1's o matmuls (whose exps
            # finished a full block ago -> PE never waits on ACT) plus one
            # filler group; a drain block finishes head 7.
            def norm(h, o_ps):
                jq, row = h // 2, 64 * (h % 2)
                zscr = trans.tile([1, S], f32, tag="zscr", bufs=2, name=f"zs{h}")
                zinv = trans.tile([1, S], f32, tag="zinv", bufs=2, name=f"zi{h}")
                zb = trans.tile([64, S], f32, tag="zb", bufs=2, name=f"zb{h}")
                nc.vector.tensor_copy(zscr[0:1, :], o_ps[64:65, :])
                nc.vector.reciprocal_approx_fast(zinv[0:1, :], zscr[0:1, :])
                nc.gpsimd.partition_broadcast(zb[0:64, :], zinv[0:1, :])
                nc.vector.tensor_mul(pair[jq][row:row + 64, :],
                                     o_ps[0:64, :], zb[0:64, :])

            merge = lambda j: nc.vector.tensor_add(pair[j][:], pair[j][:],
                                                   ctokT[j][:])
            fill_sched = {
                0: [lambda t=t: emit_v(t) for t in range(8)],
                1: [lambda tc=tc: emit_kq(kT, wk_v, tok, 1, tc) for tc in (0, 1)]
                 + [lambda tc=tc: emit_kq(qT, wq_v, ctokT, 1, tc) for tc in (0, 1)],
                2: [lambda tc=tc: emit_kq(kT, wk_v, tok, 2, tc) for tc in (0, 1)]
                 + [lambda tc=tc: emit_kq(qT, wq_v, ctokT, 2, tc) for tc in (0, 1)],
                3: [lambda tc=tc: emit_kq(kT, wk_v, tok, 3, tc) for tc in (0, 1)]
                 + [lambda tc=tc: emit_kq(qT, wq_v, ctokT, 3, tc) for tc in (0, 1)],
                4: [lambda: emit_cc(0)],
                5: [lambda: emit_cc(1)],
                8: [lambda: emit_cc(2), lambda: emit_cc(3)],
            }
            pts = {}
            o_tiles = {}

            def emit_sc(h, kt):
                jq, row = h // 2, 64 * (h % 2)
                pt = trans.tile([128, S], f16, tag="pt", bufs=12,
                                name=f"pt{h}_{kt}")
                for qc in range(2):
                    sc = sct()
                    nc.tensor.matmul(
                        sc[:],
                        kT[jq][row:row + 64, 128 * kt:128 * (kt + 1)],
                        qT[jq][row:row + 64, 512 * qc:512 * (qc + 1)],
                        start=True, stop=True)
                    nc.scalar.activation(pt[:, 512 * qc:512 * (qc + 1)], sc[:],
                                         EXP, bias=ebias[:, 0:1], scale=SCALE)
                pts[(h, kt)] = pt

            def emit_o(h, kt):
                if kt == 0:
                    o_tiles[h] = pst("o", [65, S], 2)
                o_ps = o_tiles[h]
                for qc in range(2):
                    nc.tensor.matmul(
                        o_ps[:, 512 * qc:512 * (qc + 1)],
                        vv(kt)[:, h, :],
                        pts[(h, kt)][:, 512 * qc:512 * (qc + 1)],
                        start=(kt == 0), stop=(kt == 7))
                if kt == 7:
                    del pts[(h, kt)]

            # proj groups open with the three already-merged pairs and are
            # closed by pair3 after the final merge (tail shortening)
            proj_acc = {}

            def emit_proj_partial(t):
                acc = sct()
                proj_acc[t] = acc
                for j in range(3):
                    nc.tensor.matmul(acc[:],
                                     pair[j][:, 128 * t:128 * (t + 1)],
                                     wproj_v[:, j, :],
                                     start=(j == 0), stop=False)

            def emit_proj_final(t):
                acc = proj_acc.pop(t)
                nc.tensor.matmul(acc[:], pair[3][:, 128 * t:128 * (t + 1)],
                                 wproj_v[:, 3, :], start=False, stop=True)
                nc.scalar.copy(s_sb[t][:], acc[:])

            for h in range(NH + 1):
                if h == 4:
                    # all qT consumers of ctokT are done: scale in place; each
                    # component core contributes a quarter of the ctok term
                    for j in range(4):
                        nc.vector.tensor_scalar_mul(ctokT[j][:], ctokT[j][:],
                                                    0.25)
                if h == 5:
                    merge(0)
                    merge(1)
                if h == 7:
                    merge(2)
                fl = list(fill_sched.get(h, []))
                for kt in range(8):
                    if h < NH:
                        emit_sc(h, kt)
                    if h > 0:
                        emit_o(h - 1, kt)
                    if fl and (h == 0 or kt % 2 == 1) and h != 8:
                        fl.pop(0)()
                    if h == 7 and kt in (5, 7):
                        emit_proj_partial((kt - 5) // 2)
                if h > 0:
                    norm(h - 1, o_tiles.pop(h - 1))
                if h == 8:
                    # drain block: cf-conv quarters + early proj groups cover
                    # the exposed h7 normalization chain
                    for f in fl:
                        f()
                    emit_proj_partial(2)
                    emit_proj_partial(3)
            merge(3)

            # ---- proj tail (s chunks stay in SBUF; parity-ordered rows) ----
            def emit_proj(t):
                acc = sct()
                for j in range(4):
                    nc.tensor.matmul(acc[:],
                                     pair[j][:, 128 * t:128 * (t + 1)],
                                     wproj_v[:, j, :],
                                     start=(j == 0), stop=(j == 3))
                nc.scalar.copy(s_sb[t][:], acc[:])

            def emit_cs(oc, pc):
                acc = sct()
                for k in range(4):
                    nc.tensor.matmul(acc[:],
                                     wcs_v[:, k, 128 * oc:128 * (oc + 1)],
                                     s_sb[4 * pc + k][:],
                                     start=(k == 0), stop=(k == 3))
                nc.vector.tensor_add(out_sb[oc][:, 512 * pc:512 * (pc + 1)],
                                     acc[:],
                                     outcf[oc][:, 512 * pc:512 * (pc + 1)])
                nc.sync.dma_start(
                    out_p[128 * oc:128 * (oc + 1), 512 * pc:512 * (pc + 1)],
                    out_sb[oc][:, 512 * pc:512 * (pc + 1)])

            for t in range(4):
                emit_proj_final(t)
            for t in range(4, 8):
                emit_proj(t)
                emit_cs(t - 4, 0)
            for oc in range(4):
                emit_cs(oc, 1)

    nc.compile()
    _CACHE["nc"] = nc
    return nc


def _shard_inputs(content_feat, components, pos_emb, Wq, Wkv, Wproj, bproj,
                  Wconv, bconv):
    h = np.float16
    ca = np.ascontiguousarray
    posT = ca(pos_emb.reshape(S, C).T, dtype=h)
    wk = ca(Wkv[:, :C], dtype=h)
    wv = ca(Wkv[:, C:], dtype=h)
    wq = ca(Wq, dtype=h)
    wp = ca(Wproj, dtype=h)
    wcT = ca(Wconv.T, dtype=h)
    wcs = ca(wcT[:C])
    b2 = np.stack([bconv / 4.0, Wconv[:, :C].sum(axis=1)]).astype(h)
    r2 = np.stack([np.ones(512), bproj / 4.0]).astype(h)
    in_maps = []
    for core in range(N_CORES):
        b, n = core // 4, core % 4
        in_maps.append({
            "compT": ca(components[n, b].reshape(S, C).T, dtype=h),
            "posT": posT,
            "cfT": ca(content_feat[b].reshape(S, C).T, dtype=h),
            "cfc1": ca(content_feat[b].reshape(C, S)[128 * n:128 * (n + 1)],
                       dtype=h),
            "wk": wk, "wv": wv, "wq": wq, "wproj": wp,
            "wcs": wcs,
            "wccf1": ca(wcT[C + 128 * n:C + 128 * (n + 1)]),
            "b2": b2, "r2": r2,
        })
    return in_maps


def _run(trace=False, **inputs):
    from concourse.bass_utils import run_bass_kernel_spmd

    nc = _build()
    in_maps = _shard_inputs(**inputs)
    res = run_bass_kernel_spmd(nc, in_maps, list(range(N_CORES)), trace=trace)
    outs = [res.results[i]["out_p"].astype(np.float32) for i in range(N_CORES)]
    out = np.stack([outs[0] + outs[1] + outs[2] + outs[3],
                    outs[4] + outs[5] + outs[6] + outs[7]], axis=0)
    return out.reshape(B, C, H, W).astype(np.float32), res


def kernel(**inputs):
    out, _ = _run(trace=False, **inputs)
    return out
